# revision 51
# baseline (speedup 1.0000x reference)
"""Trainium2 Bass kernel: autoregressive graph generator (GNN encoder + LSTM + GNN decoder).

Sharding: 8-way tensor parallel over the LSTM hidden/gate dim. Each core holds
1/8 of the gate rows of W_hh (bf16, SBUF-resident) and computes its slice of the
gates; h is AllGathered (bf16) every step. The encoder SAGEConv (NF=10 -> H=2048)
composed with W_ih factors through a rank-20 bottleneck, so W_ih @ W_enc_{l,r} is
precomposed on the host and the whole x-side becomes a K=20 GEMM per step.
The mean aggregation is a fixed dense matrix A built from edge_index on the host.

All layouts on device are "T-layout": [feature/hidden dim (partitions), nodes (free)].
"""

import numpy as np
import ml_dtypes

import concourse.mybir as mybir
import concourse.tile as tile
from concourse import bacc, bass_utils
from concourse.bass import ts
from concourse.masks import make_identity

BF = ml_dtypes.bfloat16
F8 = ml_dtypes.float8_e4m3

N, NF, H, NG, K = 256, 10, 2048, 20, 10
NCORES = 8
HS = H // NCORES          # 256 hidden dims per core
GD = 4 * HS               # 1024 gate rows per core
MT = GD // 128            # 8 gate m-tiles per core
KT = H // 128             # 16 h k-tiles
NT = N // 128             # 2 node tiles
GEN = NG - K              # 10 generated steps

_PROG = [None]


def _emit_decoder_tail(nc, pools, consts, t, vw_ps):
    """Gen-step decoder tail (after the v' GEMM): x_pred = A@v + w + b,
    x_next = [static2 | x_pred]; returns (m10, x10) bf16 tiles [10, N] and
    DMAs x_next to the output."""
    f32, bf16 = mybir.dt.float32, mybir.dt.bfloat16
    cpool, wpool, apool, gpool, spool = pools
    at, qr, st2, ident, out_d = (
        consts["at"], consts["qr"], consts["st2"],
        consts["ident"], consts["out_d"],
    )
    s = t - K
    vw_sb = wpool.tile([16, N], bf16, tag="vw", name=f"vwsb{t}")
    nc.vector.tensor_scalar_add(vw_sb[:], vw_ps[:], qr[:, s:s + 1])

    # transpose v'|w' -> non-T [N, 16] per node-tile
    vwT = []
    for j in range(NT):
        tp = spool.tile([128, 16], bf16, tag="sp", name=f"vwT{t}_{j}")
        nc.tensor.transpose(tp[:], vw_sb[:, ts(j, 128)], ident[:16, :16])
        tpsb = wpool.tile([128, 16], bf16, tag=f"vwTs{j}", name=f"vwTs{t}_{j}")
        nc.vector.tensor_copy(tpsb[:], tp[:])
        vwT.append(tpsb)

    # xa = A @ v  (per output node tile), x_next = [st2 | xa + w]
    xnext, xnb = [], []
    for j in range(NT):
        xa = spool.tile([128, 8], f32, tag="sp", name=f"xa{t}_{j}")
        for kk in range(NT):
            nc.tensor.matmul(xa[:], at[kk][:, ts(j, 128)], vwT[kk][:, 0:8],
                             start=(kk == 0), stop=(kk == NT - 1))
        xn = wpool.tile([128, NF], f32, tag=f"xn{j}", name=f"xn{t}_{j}")
        nc.vector.tensor_copy(xn[:, 0:2], st2[j][:])
        nc.vector.tensor_add(xn[:, 2:NF], xa[:], vwT[j][:, 8:16])
        xb = wpool.tile([128, NF], bf16, tag=f"xnb{j}", name=f"xnb{t}_{j}")
        nc.vector.tensor_copy(xb[:], xn[:])
        nc.sync.dma_start(out_d[s, ts(j, 128), :], xn[:])
        xnext.append(xn)
        xnb.append(xb)

    # m10 = (A @ x_next).T  [10, N];  x10 = x_next.T  [10, N]  (both bf16)
    m10 = wpool.tile([NF, N], bf16, tag="m10", name=f"m10_{t}")
    mp = spool.tile([NF, N], f32, tag="sp", name=f"mp{t}")
    for kk in range(NT):
        nc.tensor.matmul(mp[:], xnb[kk][:], at[kk][:],
                         start=(kk == 0), stop=(kk == NT - 1))
    nc.vector.tensor_copy(m10[:], mp[:])
    x10 = wpool.tile([NF, N], bf16, tag="x10", name=f"x10_{t}")
    for kk in range(NT):
        xt = spool.tile([NF, 128], bf16, tag="sp", name=f"xt{t}_{kk}")
        nc.tensor.transpose(xt[:], xnb[kk][:], ident[:])
        nc.vector.tensor_copy(x10[:, ts(kk, 128)], xt[:])
    return m10, x10


def _emit_step(nc, pools, consts, t, h_tiles, c_prev, dpool):
    """One LSTM step: gate GEMMs + cell update + pipelined 2-phase AllGather.

    hbig layout (and host weight packing) orders k-tiles [all hh=0 slices |
    all hh=1 slices], so DoubleRow pairs j<4 depend only on AllGather-A
    (which launches while the hh=1 elementwise is still running) and pairs
    j>=4 only on AllGather-B.
    Returns (c_new, hbig_for_next_step).
    """
    f32, bf16 = mybir.dt.float32, mybir.dt.bfloat16
    fp8 = mybir.dt.float8e4
    cpool, wpool, apool, gpool, spool = pools
    whh, wc, bias, r20w = consts["whh"], consts["wc"], consts["bias"], consts["r20w"]
    wdec = consts["wdec"]
    Sig = mybir.ActivationFunctionType.Sigmoid
    Tanh = mybir.ActivationFunctionType.Tanh
    have_h = t > 0
    m_order = [0, 2, 4, 6, 1, 3, 5, 7]  # finish hidden-half 0 (i,f,g,o) early
    DR = mybir.MatmulPerfMode.DoubleRow

    hview = h_tiles[:].rearrange("p (a n) -> p a n", a=KT) if have_h else None

    if t < K:
        xparts = [(wc, r20w[:, t * N:(t + 1) * N])]
    else:
        vw_ps = spool.tile([16, N], f32, tag="sp", name=f"vwps{t}")
        for j in range(KT // 2):
            nc.tensor.matmul(vw_ps[:],
                             wdec[j][:].rearrange("p (s w) -> p s w", s=2),
                             hview[:, 2 * j:2 * j + 2, :],
                             start=(j == 0), stop=(j == KT // 2 - 1),
                             perf_mode=DR)
        m10, x10 = _emit_decoder_tail(nc, pools, consts, t, vw_ps)
        xparts = [(wc[0:20], m10[:]), (consts["wc2"], x10[:])]

    gp = {}
    for m in m_order:
        g = gpool.tile([128, N], f32, tag="gp", name=f"gp{t}_{m}")
        for pi, (wmat, rhs) in enumerate(xparts):
            nc.tensor.matmul(g[:], wmat[0:rhs.shape[0], ts(m, 128)], rhs,
                             start=(pi == 0),
                             stop=(pi == len(xparts) - 1) and not have_h)
        if have_h:
            for j in range(KT // 2):
                nc.tensor.matmul(
                    g[:],
                    whh[j][:].rearrange("p (s w) -> p s w", s=2)[:, :, ts(m, 128)],
                    hview[:, 2 * j:2 * j + 2, :],
                    start=False, stop=(j == KT // 2 - 1),
                    perf_mode=DR)
        gp[m] = g

    def gsl(m):
        return gp[m][:]

    h2 = apool.tile([128, 2 * N], fp8, tag="h2", name=f"h2_{t}")
    hb = None
    if t < NG - 1:
        hb = wpool.tile([128, KT * N], fp8, tag="hbig", name=f"hbig{t}")
    c_new = []
    for hh in range(2):
        def bcol(m):
            return bias[:, (m * NG + t):(m * NG + t + 1)]
        si = apool.tile([128, N], f32, tag="si", name=f"si{t}_{hh}")
        nc.scalar.activation(si[:], gsl(0 + hh), Sig, bias=bcol(0 + hh))
        sf = apool.tile([128, N], f32, tag="sf", name=f"sf{t}_{hh}")
        nc.scalar.activation(sf[:], gsl(2 + hh), Sig, bias=bcol(2 + hh))
        tg = apool.tile([128, N], f32, tag="tg", name=f"tg{t}_{hh}")
        nc.scalar.activation(tg[:], gsl(4 + hh), Tanh, bias=bcol(4 + hh))
        so = apool.tile([128, N], f32, tag="so", name=f"so{t}_{hh}")
        nc.scalar.activation(so[:], gsl(6 + hh), Sig, bias=bcol(6 + hh))

        cn = wpool.tile([128, N], f32, tag=f"c{hh}", name=f"c{t}_{hh}")
        if t == 0:
            nc.vector.tensor_mul(cn[:], si[:], tg[:])          # c = sig(i)*tanh(g)
        else:
            p = apool.tile([128, N], f32, tag="p", name=f"p{t}_{hh}")
            nc.vector.tensor_mul(p[:], si[:], tg[:])
            tmp = apool.tile([128, N], f32, tag="tmp", name=f"tmp{t}_{hh}")
            nc.vector.tensor_mul(tmp[:], sf[:], c_prev[hh][:])
            nc.vector.tensor_add(cn[:], tmp[:], p[:])
        tc2 = apool.tile([128, N], f32, tag="tc", name=f"tc{t}_{hh}")
        nc.scalar.activation(tc2[:], cn[:], Tanh)
        nc.vector.tensor_mul(h2[:, ts(hh, N)], so[:], tc2[:])
        c_new.append(cn)

    if t < NG - 1:
        inb = dpool.tile([N, N], fp8, tag="inb", name=f"inb{t}")
        outb = dpool.tile([H, N], fp8, tag="outb", name=f"outb{t}")
        # per-half input DMAs: the hh=0 half ships while the hh=1
        # elementwise is still running, shortening the collective's
        # input-wait phase
        for hh in range(2):
            nc.sync.dma_start(inb[ts(hh, 128), :], h2[:, ts(hh, N)])
        nc.gpsimd.collective_compute(
            "AllGather",
            mybir.AluOpType.bypass,
            replica_groups=[list(range(NCORES))],
            ins=[inb.opt()],
            outs=[outb.opt()],
        )
        outb3 = outb.rearrange("(a p) n -> p a n", p=128)
        for half in range(2):
            nc.sync.dma_start(
                hb[:, ts(half, KT * N // 2)].rearrange(
                    "p (a n) -> p a n", a=KT // 2),
                outb3[:, ts(half, KT // 2), :])
    return c_new, hb


def _build_program():
    f32, bf16 = mybir.dt.float32, mybir.dt.bfloat16
    nc = bacc.Bacc("TRN2", target_bir_lowering=False, debug=False,
                   num_devices=NCORES)

    fp8 = mybir.dt.float8e4
    whhT_d = nc.dram_tensor("whhT", [H // 2, 2 * GD], fp8,
                            kind="ExternalInput").ap()
    wcT_d = nc.dram_tensor("wcT", [20, GD], bf16, kind="ExternalInput").ap()
    wc2T_d = nc.dram_tensor("wc2T", [NF, GD], bf16, kind="ExternalInput").ap()
    bias_d = nc.dram_tensor("biases", [128, MT * NG], f32, kind="ExternalInput").ap()
    at_d = nc.dram_tensor("at", [N, N], bf16, kind="ExternalInput").ap()
    wdec_d = nc.dram_tensor("wdecT", [H // 2, 32], fp8,
                            kind="ExternalInput").ap()
    qr_d = nc.dram_tensor("qr", [16, GEN], f32, kind="ExternalInput").ap()
    r20_d = nc.dram_tensor("rhs20w", [20, K * N], bf16, kind="ExternalInput").ap()
    st2_d = nc.dram_tensor("st2", [N, 2], f32, kind="ExternalInput").ap()
    out_d = nc.dram_tensor("gen", [GEN, N, NF], f32, kind="ExternalOutput").ap()

    with tile.TileContext(nc) as tc:
        with (
            tc.tile_pool(name="const", bufs=1) as cpool,
            tc.tile_pool(name="work", bufs=2) as wpool,
            tc.tile_pool(name="act", bufs=3) as apool,
            tc.tile_pool(name="gp", bufs=5, space="PSUM") as gpool,
            tc.tile_pool(name="sp", bufs=3, space="PSUM") as spool,
            tc.tile_pool(name="dram", bufs=2, space="DRAM") as dpool,
        ):
            pools = (cpool, wpool, apool, gpool, spool)

            fp8 = mybir.dt.float8e4
            whh = []
            for k in range(KT // 2):
                w = cpool.tile([128, 2 * GD], fp8, tag=f"whh{k}", name=f"whh{k}")
                nc.sync.dma_start(w[:], whhT_d[ts(k, 128), :])
                whh.append(w)
            wc = cpool.tile([20, GD], bf16, tag="wc", name="wc")
            nc.sync.dma_start(wc[:], wcT_d[:])
            wc2 = cpool.tile([NF, GD], bf16, tag="wc2", name="wc2")
            nc.sync.dma_start(wc2[:], wc2T_d[:])
            at = []
            for k in range(NT):
                a = cpool.tile([128, N], bf16, tag=f"at{k}", name=f"at{k}")
                nc.sync.dma_start(a[:], at_d[ts(k, 128), :])
                at.append(a)
            wdec = []
            for k in range(KT // 2):
                w = cpool.tile([128, 32], fp8, tag=f"wdec{k}", name=f"wdec{k}")
                nc.sync.dma_start(w[:], wdec_d[ts(k, 128), :])
                wdec.append(w)
            bias = cpool.tile([128, MT * NG], f32, tag="bias", name="bias")
            nc.sync.dma_start(bias[:], bias_d[:])
            qr = cpool.tile([16, GEN], f32, tag="qr", name="qr")
            nc.sync.dma_start(qr[:], qr_d[:])
            r20w = cpool.tile([20, K * N], bf16, tag="r20w", name="r20w")
            nc.sync.dma_start(r20w[:], r20_d[:])
            st2 = []
            for j in range(NT):
                s = cpool.tile([128, 2], f32, tag=f"st2{j}", name=f"st2_{j}")
                nc.sync.dma_start(s[:], st2_d[ts(j, 128), :])
                st2.append(s)
            ident = cpool.tile([128, 128], bf16, tag="ident", name="ident")
            make_identity(nc, ident[:])

            consts = dict(whh=whh, wc=wc, wc2=wc2, bias=bias, at=at, wdec=wdec,
                          qr=qr, r20w=r20w, st2=st2, ident=ident, out_d=out_d)

            h_tiles, c_prev = None, None
            for t in range(NG):
                c_prev, h_tiles = _emit_step(nc, pools, consts, t, h_tiles,
                                             c_prev, dpool)
    nc.compile()
    return nc


def _host_tensors(inputs):
    """All host-side preprocessing: A matrix, weight composition, per-core shards."""
    f32 = np.float32
    kg = np.asarray(inputs["known_graphs"], f32)
    ei = np.asarray(inputs["edge_index"])
    W_enc_l = np.asarray(inputs["W_enc_l"], f32)
    b_enc_l = np.asarray(inputs["b_enc_l"], f32)
    W_enc_r = np.asarray(inputs["W_enc_r"], f32)
    pos = np.asarray(inputs["pos_emb"], f32)
    W_ih = np.asarray(inputs["W_ih"], f32)
    W_hh = np.asarray(inputs["W_hh"], f32)
    b_ih = np.asarray(inputs["b_ih"], f32)
    b_hh = np.asarray(inputs["b_hh"], f32)
    W_dec_l = np.asarray(inputs["W_dec_l"], f32)
    b_dec_l = np.asarray(inputs["b_dec_l"], f32)
    W_dec_r = np.asarray(inputs["W_dec_r"], f32)

    src, dst = np.asarray(ei[0]), np.asarray(ei[1])
    C = np.zeros((N, N), np.float64)
    np.add.at(C, (dst, src), 1.0)
    cnt = C.sum(1)
    A = (C / np.maximum(cnt, 1.0)[:, None]).astype(f32)

    c64 = np.float64
    Wc1 = W_ih.astype(c64) @ W_enc_l.astype(c64)          # [4H, NF]
    Wc2 = W_ih.astype(c64) @ W_enc_r.astype(c64)
    Wc = np.concatenate([Wc1, Wc2], 1)                    # [4H, 20]
    # bias_t = W_ih @ (b_enc_l + pe_t) + b_ih + b_hh  -> [NG, 4H]
    bias_all = (W_ih.astype(c64) @ (b_enc_l.astype(c64)[:, None] + pos.astype(c64).T)).T \
        + b_ih.astype(c64) + b_hh.astype(c64)
    bias_all = bias_all.astype(f32)
    # decoder pe folds: [16, GEN]
    qr = np.concatenate([
        (pos[K:NG].astype(c64) @ W_dec_l.T.astype(c64)).T,
        (pos[K:NG].astype(c64) @ W_dec_r.T.astype(c64)).T
        + b_dec_l.astype(c64)[:, None],
    ], 0).astype(f32)

    # warm-up rhs20: [20, K*N], col index t*N + i
    mean_w = np.einsum("ij,tjf->tif", A.astype(c64), kg.astype(c64))  # [K, N, NF]
    r20w = np.concatenate([
        np.transpose(mean_w, (2, 0, 1)).reshape(NF, -1),
        np.transpose(kg.astype(c64), (2, 0, 1)).reshape(NF, -1),
    ], 0).astype(f32)

    # DoubleRow pair packing: [KT/2 * 128, 2*cols], row j*128+p holds
    # global k-tiles (2j, 2j+1) side by side along the free dim
    def pack_pairs(wT):  # wT [H, cols] -> [H/2, 2*cols]
        cols = wT.shape[1]
        return np.ascontiguousarray(
            wT.reshape(KT // 2, 2, 128, cols).transpose(0, 2, 1, 3)
            .reshape(H // 2, 2 * cols))

    wdecT = np.concatenate([W_dec_l, W_dec_r], 0).T        # [H, 16]
    shared = {
        "at": np.ascontiguousarray(A.T).astype(BF),
        "wdecT": pack_pairs(wdecT).astype(F8),
        "qr": np.ascontiguousarray(qr),
        "rhs20w": np.ascontiguousarray(r20w).astype(BF),
        "st2": np.ascontiguousarray(kg[-1, :, :2]),
    }

    in_maps = []
    for c in range(NCORES):
        idx = np.concatenate([np.arange(g * H + c * HS, g * H + (c + 1) * HS)
                              for g in range(4)])
        whhT = pack_pairs(W_hh[idx, :].T).astype(F8)                  # [H/2, 2GD]
        wcT = np.ascontiguousarray(Wc[idx, :].T).astype(BF)           # [20, GD]
        wc2T = np.ascontiguousarray(Wc[idx, NF:].T).astype(BF)        # [NF, GD]
        bc = bias_all[:, idx].T                                       # [GD, NG]
        bt = np.ascontiguousarray(
            bc.reshape(MT, 128, NG).transpose(1, 0, 2).reshape(128, MT * NG))
        in_maps.append({
            "whhT": whhT, "wcT": wcT, "wc2T": wc2T, "biases": bt, **shared,
        })
    return in_maps


def kernel(**inputs):
    if _PROG[0] is None:
        _PROG[0] = _build_program()
    nc = _PROG[0]
    in_maps = _host_tensors(inputs)
    res = bass_utils.run_bass_kernel_spmd(
        nc, in_maps, core_ids=list(range(NCORES)))
    return np.ascontiguousarray(res.results[0]["gen"]).astype(np.float32)


# exposed for test.py profiling
def run_profiled(inputs, **kwargs):
    if _PROG[0] is None:
        _PROG[0] = _build_program()
    in_maps = _host_tensors(inputs)
    return bass_utils.run_bass_kernel_spmd(
        _PROG[0], in_maps, core_ids=list(range(NCORES)), **kwargs)
